# revision 3
# baseline (speedup 1.0000x reference)
"""AttentiveFP readout kernel v2 for 8 Trainium2 NeuronCores.

Graph-contiguous sharding of V=500k nodes over 8 cores (no collectives).
Per core: 128-graph tiles; nodes of a tile live in SBUF once (bf16).

Key design vs v1:
- Transposed ("feature-on-partition") graph state gfT [2x128f, 128g]:
  all GRU/projection matmuls take lhsT=weightT chunks, rhs=state chunks;
  no per-timestep PE transposes or PSUM->SBUF copies.
- Windowed one-hot segment matmuls: node subtile s only spans graphs
  [o(s), o(s)+W) for a *global* offset table o(s) (tiles start at exact
  graph boundaries, so drift is bounded); rhs = Mn[:, s, :] is [128, W]
  so each matmul streams W columns instead of 128/257.
- Per-node logit w = x . wl_n computed on the TensorEngine from a host
  pre-transposed fp8(e4m3) copy of node features (N=2 matmuls).
- u = relu(g) . wl_g broadcast to nodes via gpsimd ap_gather (indices
  depend only on the free position -> SPMD-safe, shipped as data).
- Biases folded into matmul accumulations via K=1 ones-row matmuls;
  ELU's "-1" folded into the GRU input bias on the host.
"""

import numpy as np
from contextlib import ExitStack

import concourse.bass as bass
import concourse.bacc as bacc
import concourse.mybir as mybir
from concourse import tile
from concourse.bass_utils import run_bass_kernel_spmd

F32 = mybir.dt.float32
BF16 = mybir.dt.bfloat16
FP8 = mybir.dt.float8e4
I16 = mybir.dt.int16
NP_BF16 = mybir.dt.np(mybir.dt.bfloat16)
NP_FP8 = mybir.dt.np(mybir.dt.float8e4)
AOP = mybir.AluOpType
ACT = mybir.ActivationFunctionType
AX = mybir.AxisListType

NCORES = 8
F = 256
T = 2
W = 24  # graph window per 128-node subtile (max observed span ~18)
LAST_RESULT = None


def _build_program(NT_G, NSUB, offs, use_lrelu=False):
    ctx = ExitStack()
    nc = bacc.Bacc("TRN2")
    clampc = nc.alloc_sbuf_tensor("const-f32-clamp", [128, 1], F32)
    nc.gpsimd.memset(clampc.ap(), 30.0)
    epsc = nc.alloc_sbuf_tensor("const-f32-eps", [128, 1], F32)
    nc.gpsimd.memset(epsc.ap(), 1e-6)
    zeroc = nc.alloc_sbuf_tensor("const-f32-zero", [128, 1], F32)
    nc.gpsimd.memset(zeroc.ap(), 0.0)
    nc.all_engine_barrier()

    NPAD = 1 if (NSUB + 1) * W % 16 == 0 else 2
    NIDX = (NSUB + NPAD) * W  # ap_gather num_idxs (multiple of 16)
    assert NIDX % 16 == 0

    nf_d = nc.dram_tensor("nf", [NT_G * 128, NSUB * F], BF16, kind="ExternalInput")
    nt8_d = nc.dram_tensor("nt8", [NT_G * 128, 2 * NSUB * 128], FP8, kind="ExternalInput")
    segw_d = nc.dram_tensor("segw", [NT_G * 128, NSUB], BF16, kind="ExternalInput")
    gidx_d = nc.dram_tensor("gidx", [128, NIDX // 16], I16, kind="ExternalInput")
    iotaw_d = nc.dram_tensor("iotaw", [128, W], BF16, kind="ExternalInput")
    onesr_d = nc.dram_tensor("onesr", [1, 128], BF16, kind="ExternalInput")
    onescol_d = nc.dram_tensor("onescol", [128, 1], BF16, kind="ExternalInput")
    zrow_d = nc.dram_tensor("zrow", [1, 128], BF16, kind="ExternalInput")
    zcol128_d = nc.dram_tensor("zcol128", [1, 128], BF16, kind="ExternalInput")
    wl8_d = nc.dram_tensor("wl8", [128, 2 * 2], FP8, kind="ExternalInput")
    wlg_d = nc.dram_tensor("wlg", [128, 2 * 2], BF16, kind="ExternalInput")
    blrow_d = nc.dram_tensor("blrow", [1, 2 * 128], BF16, kind="ExternalInput")
    wpt_d = [nc.dram_tensor(f"wpt{t}", [128, 4 * 128], BF16, kind="ExternalInput") for t in range(T)]
    bpc_d = [nc.dram_tensor(f"bpc{t}", [128, 2], F32, kind="ExternalInput") for t in range(T)]
    wih_d = [nc.dram_tensor(f"wih{t}", [128, 12 * 128], BF16, kind="ExternalInput") for t in range(T)]
    whh_d = [nc.dram_tensor(f"whh{t}", [128, 12 * 128], BF16, kind="ExternalInput") for t in range(T)]
    brzc_d = [nc.dram_tensor(f"brzc{t}", [128, 4], F32, kind="ExternalInput") for t in range(T)]
    bnxc_d = [nc.dram_tensor(f"bnxc{t}", [128, 2], F32, kind="ExternalInput") for t in range(T)]
    bnhc_d = [nc.dram_tensor(f"bnhc{t}", [128, 2], F32, kind="ExternalInput") for t in range(T)]
    out_d = nc.dram_tensor("out", [NT_G * 2 * 128, 128], F32, kind="ExternalOutput")

    with tile.TileContext(nc) as tc:
      with tc.sbuf_pool(name="const", bufs=1) as cpool, \
           tc.sbuf_pool(name="nfp", bufs=3) as nfpool, \
           tc.sbuf_pool(name="ntp", bufs=3) as ntpool, \
           tc.sbuf_pool(name="mn", bufs=3) as mnpool, \
           tc.sbuf_pool(name="small", bufs=3) as spool, \
           tc.sbuf_pool(name="scr", bufs=3) as scrpool, \
           tc.psum_pool(name="pbig", bufs=1) as pbig, \
           tc.psum_pool(name="prz", bufs=1) as przp, \
           tc.psum_pool(name="prow", bufs=1) as prow, \
           tc.psum_pool(name="ptiny", bufs=1) as ptiny:

        gidx_sb = cpool.tile_from(gidx_d[:, :], name="gidx_sb")
        iotaw_sb = cpool.tile_from(iotaw_d[:, :], name="iotaw_sb")
        onesr_sb = cpool.tile_from(onesr_d[:, :], name="onesr_sb")
        onescol_sb = cpool.tile_from(onescol_d[:, :], name="onescol_sb")
        zrow_sb = cpool.tile_from(zrow_d[:, :], name="zrow_sb")
        zcol_sb = cpool.tile_from(zcol128_d[:, :], name="zcol_sb")
        wl8_sb = cpool.tile([128, 2, 2], FP8, name="wl8_sb")
        nc.sync.dma_start(wl8_sb[:, :, :], wl8_d[:, :])
        wlg_sb = cpool.tile([128, 2, 2], BF16, name="wlg_sb")
        nc.sync.dma_start(wlg_sb[:, :, :], wlg_d[:, :])
        blrow_sb = cpool.tile_from(blrow_d[:, :], name="blrow_sb")
        wpt_sb = [cpool.tile_from(wpt_d[t][:, :], name=f"wpt_sb{t}") for t in range(T)]
        bpc_sb = [cpool.tile_from(bpc_d[t][:, :], name=f"bpc_sb{t}") for t in range(T)]
        wih_sb = [cpool.tile_from(wih_d[t][:, :], name=f"wih_sb{t}") for t in range(T)]
        whh_sb = [cpool.tile_from(whh_d[t][:, :], name=f"whh_sb{t}") for t in range(T)]
        brzc_sb = [cpool.tile_from(brzc_d[t][:, :], name=f"brzc_sb{t}") for t in range(T)]
        bnxc_sb = [cpool.tile_from(bnxc_d[t][:, :], name=f"bnxc_sb{t}") for t in range(T)]
        bnhc_sb = [cpool.tile_from(bnhc_d[t][:, :], name=f"bnhc_sb{t}") for t in range(T)]

        def tile_work(j):
            r0 = j * 128
            nf = nfpool.tile([128, NSUB * F], BF16, name=f"nf_{j}", tag="nf")
            nc.sync.dma_start(nf[:, :], nf_d[r0:r0 + 128, :])
            nt8 = ntpool.tile([128, 2, NSUB * 128], FP8, name=f"nt8_{j}", tag="nt8")
            nc.sync.dma_start(nt8[:, :, :], nt8_d[r0:r0 + 128, :])
            segw = spool.tile([128, NSUB], BF16, name=f"segw_{j}", tag="segw")
            nc.sync.dma_start(segw[:, :], segw_d[r0:r0 + 128, :])
            yield

            # Mn[p, s, w] = (segw[p, s] == w), bf16 one-hot within window
            Mn = mnpool.tile([128, NSUB, W], BF16, name=f"mn_{j}", tag="mn")
            nc.vector.tensor_tensor(
                Mn[:, :, :],
                segw[:, :].unsqueeze(2).broadcast_to((128, NSUB, W)),
                iotaw_sb[:, :].unsqueeze(1).broadcast_to((128, NSUB, W)),
                op=AOP.is_equal)
            yield

            # w01[v, t] = x_v . wl_n[t]  (fp8 transposed copy, N=2 matmuls)
            w01ps = ptiny.tile([128, NSUB, 2], F32, name=f"w01ps_{j}", tag="tiny")
            for s in range(NSUB):
                for k in range(2):
                    nc.tensor.matmul(w01ps[:, s, :],
                                     nt8[:, k, s * 128:(s + 1) * 128],
                                     wl8_sb[:, k, :],
                                     start=(k == 0), stop=(k == 1))
            w01 = spool.tile([128, NSUB, 2], F32, name=f"w01_{j}", tag="w01")
            nc.vector.tensor_copy(w01[:, :, :], w01ps[:, :, :])
            yield

            # initial segment sum, transposed: g0T[f-chunk, g]
            g0T = pbig.tile([128, 2, 128], F32, name=f"g0T_{j}", tag="st", bufs=2)
            for k in range(2):
                nc.tensor.matmul(g0T[:, k, :], zcol_sb[:, :], zrow_sb[:, :],
                                 start=True, stop=False, skip_group_check=True)
            for s in range(NSUB):
                o = offs[s]
                for k in range(2):
                    nc.tensor.matmul(
                        g0T[:, k, o:o + W],
                        nf[:, s * F + k * 128: s * F + (k + 1) * 128],
                        Mn[:, s, :],
                        start=False, stop=(s == NSUB - 1 and k == 1),
                        skip_group_check=True)
            yield
            gf = spool.tile([128, 2, 128], F32, name=f"gf0_{j}", tag="gf", bufs=4)
            nc.vector.tensor_copy(gf[:, :, :], g0T[:, :, :])
            hTb = spool.tile([128, 2, 128], BF16, name=f"hTb0_{j}", tag="hTb", bufs=4)
            nc.vector.tensor_copy(hTb[:, :, :], g0T[:, :, :])

            for t in range(T):
                yield
                # u_row[g] = relu(gf) . wl_g[t] + bl[t]
                rgf = spool.tile([128, 2, 128], BF16, name=f"rgf_{j}_{t}", tag="rgf")
                nc.scalar.activation(rgf[:, :, :], gf[:, :, :], ACT.Relu)
                ups = prow.tile([1, 128], F32, name=f"ups_{j}_{t}", tag="row")
                for k in range(2):
                    nc.tensor.matmul(ups[:, :], wlg_sb[:, k, t:t + 1],
                                     rgf[:, k, :], start=(k == 0), stop=False,
                                     skip_group_check=True)
                nc.tensor.matmul(ups[:, :], onesr_sb[0:1, 0:1],
                                 blrow_sb[:, t * 128:(t + 1) * 128],
                                 start=False, stop=True, skip_group_check=True)
                # clamp to keep exp() finite on junk graphs, copy to sbuf
                urow = spool.tile([1, 128], F32, name=f"urow_{j}_{t}", tag="urow")
                nc.vector.tensor_tensor(urow[:, :], ups[:, :],
                                        clampc.ap()[0:1, :].broadcast_to((1, 128)),
                                        op=AOP.min)
                ubc = spool.tile([128, 128], F32, name=f"ubc_{j}_{t}", tag="ubc")
                nc.gpsimd.partition_broadcast(ubc[:, :], urow[:, :], channels=128)
                yield
                uwin = scrpool.tile([128, NSUB + NPAD, W], F32, name=f"uwin_{j}_{t}", tag="uwin")
                nc.gpsimd.ap_gather(uwin[:, :, :], ubc[:, :], gidx_sb[:, :],
                                    channels=128, num_elems=128, d=1,
                                    num_idxs=NIDX)
                # z = w01 + u (window layout), leaky-relu, exp
                zadd = scrpool.tile([128, NSUB, W], F32, name=f"zadd_{j}_{t}", tag="zadd")
                nc.vector.tensor_tensor(
                    zadd[:, :, :],
                    uwin[:, 0:NSUB, :],
                    w01[:, :, t:t + 1].broadcast_to((128, NSUB, W)),
                    op=AOP.add)
                ebf = scrpool.tile([128, NSUB, W], BF16, name=f"ebf_{j}_{t}", tag="ebf")
                if use_lrelu:
                    zl = scrpool.tile([128, NSUB, W], F32, name=f"zl_{j}_{t}", tag="zl")
                    nc.scalar.activation(zl[:, :, :], zadd[:, :, :], ACT.Lrelu,
                                         alpha=0.01)
                    nc.scalar.activation(ebf[:, :, :], zl[:, :, :], ACT.Exp)
                else:
                    zs = scrpool.tile([128, NSUB, W], F32, name=f"zs_{j}_{t}", tag="zs")
                    nc.scalar.mul(zs[:, :, :], zadd[:, :, :], 0.01)
                    zl = scrpool.tile([128, NSUB, W], F32, name=f"zl_{j}_{t}", tag="zl")
                    nc.vector.tensor_tensor(zl[:, :, :], zadd[:, :, :],
                                            zs[:, :, :], op=AOP.max)
                    nc.scalar.activation(ebf[:, :, :], zl[:, :, :], ACT.Exp)
                yield
                # masked exp weights
                Me = mnpool.tile([128, NSUB, W], BF16, name=f"me_{j}_{t}", tag="me")
                nc.vector.tensor_tensor(Me[:, :, :], Mn[:, :, :], ebf[:, :, :],
                                        op=AOP.mult)
                yield
                # weighted segment sum (transposed) + denominator row
                ST = pbig.tile([128, 2, 128], F32, name=f"st_{j}_{t}", tag="st", bufs=2)
                for k in range(2):
                    nc.tensor.matmul(ST[:, k, :], zcol_sb[:, :], zrow_sb[:, :],
                                     start=True, stop=False, skip_group_check=True)
                for s in range(NSUB):
                    o = offs[s]
                    for k in range(2):
                        nc.tensor.matmul(
                            ST[:, k, o:o + W],
                            nf[:, s * F + k * 128: s * F + (k + 1) * 128],
                            Me[:, s, :],
                            start=False, stop=(s == NSUB - 1 and k == 1),
                            skip_group_check=True)
                # denominator: same stationary lhsT for all subtiles, run
                # back-to-back so the PE can keep the weights loaded
                drow = prow.tile([1, 128], F32, name=f"drow_{j}_{t}", tag="row")
                nc.tensor.matmul(drow[:, :], zcol_sb[0:1, 0:1], zrow_sb[:, :],
                                 start=True, stop=False, skip_group_check=True)
                for s in range(NSUB):
                    o = offs[s]
                    nc.tensor.matmul(drow[0:1, o:o + W], onescol_sb[:, :],
                                     Me[:, s, :], start=False,
                                     stop=(s == NSUB - 1), skip_group_check=True)
                yield
                dmax = spool.tile([1, 128], F32, name=f"dmax_{j}_{t}", tag="dmax")
                nc.vector.tensor_tensor(dmax[:, :], drow[:, :],
                                        epsc.ap()[0:1, :].broadcast_to((1, 128)),
                                        op=AOP.max)
                rd = spool.tile([1, 128], F32, name=f"rd_{j}_{t}", tag="rd")
                nc.vector.reciprocal(rd[:, :], dmax[:, :])
                rdb = spool.tile([128, 128], F32, name=f"rdb_{j}_{t}", tag="rdb")
                nc.gpsimd.partition_broadcast(rdb[:, :], rd[:, :], channels=128)
                yield
                stl = spool.tile([128, 2, 128], BF16, name=f"stl_{j}_{t}", tag="stl")
                nc.vector.tensor_tensor(
                    stl[:, :, :], ST[:, :, :],
                    rdb[:, :].unsqueeze(1).broadcast_to((128, 2, 128)),
                    op=AOP.mult)
                # g_reprT = Wp @ stl + bp  (transposed; bp via activation bias)
                gr = pbig.tile([128, 2, 128], F32, name=f"gr_{j}_{t}", tag="gr")
                for m in range(2):
                    for k in range(2):
                        nc.tensor.matmul(gr[:, m, :],
                                         wpt_sb[t][:, (k * 2 + m) * 128:(k * 2 + m + 1) * 128],
                                         stl[:, k, :],
                                         start=(k == 0), stop=(k == 1),
                                         skip_group_check=True)
                yield
                # ELU (the -1 is folded into bnx on host)
                xn = scrpool.tile([128, 2, 128], F32, name=f"xn_{j}_{t}", tag="xn")
                for m in range(2):
                    nc.vector.tensor_scalar(xn[:, m, :], gr[:, m, :],
                                            bpc_sb[t][:, m:m + 1], 0.0,
                                            op0=AOP.add, op1=AOP.min)
                en = spool.tile([128, 2, 128], BF16, name=f"en_{j}_{t}", tag="en")
                nc.scalar.activation(en[:, :, :], xn[:, :, :], ACT.Exp)
                xp = spool.tile([128, 2, 128], BF16, name=f"xp_{j}_{t}", tag="xp")
                for m in range(2):
                    nc.scalar.activation(xp[:, m, :], gr[:, m, :], ACT.Relu,
                                         bias=bpc_sb[t][:, m:m + 1])
                ctxb = spool.tile([128, 2, 128], BF16, name=f"ctx_{j}_{t}", tag="ctx")
                nc.vector.tensor_tensor(ctxb[:, :, :], en[:, :, :], xp[:, :, :],
                                        op=AOP.add)
                yield
                # GRU gates, all transposed-layout matmuls
                rzps = przp.tile([128, 4, 128], F32, name=f"rz_{j}_{t}", tag="rz")
                for m in range(4):
                    mm = 0
                    for src, wt in ((ctxb, wih_sb[t]), (hTb, whh_sb[t])):
                        for k in range(2):
                            nc.tensor.matmul(rzps[:, m, :],
                                             wt[:, (k * 6 + m) * 128:(k * 6 + m + 1) * 128],
                                             src[:, k, :],
                                             start=(mm == 0), stop=(mm == 3),
                                             skip_group_check=True)
                            mm += 1
                inps = pbig.tile([128, 2, 128], F32, name=f"in_{j}_{t}", tag="in")
                hnps = pbig.tile([128, 2, 128], F32, name=f"hn_{j}_{t}", tag="hn")
                for m in range(2):
                    for k in range(2):
                        nc.tensor.matmul(inps[:, m, :],
                                         wih_sb[t][:, (k * 6 + 4 + m) * 128:(k * 6 + 5 + m) * 128],
                                         ctxb[:, k, :], start=(k == 0), stop=(k == 1),
                                         skip_group_check=True)
                        nc.tensor.matmul(hnps[:, m, :],
                                         whh_sb[t][:, (k * 6 + 4 + m) * 128:(k * 6 + 5 + m) * 128],
                                         hTb[:, k, :], start=(k == 0), stop=(k == 1),
                                         skip_group_check=True)
                yield
                eneg = spool.tile([128, 4, 128], F32, name=f"eneg_{j}_{t}", tag="eneg")
                for m in range(4):
                    nc.scalar.activation(eneg[:, m, :], rzps[:, m, :], ACT.Exp,
                                         bias=brzc_sb[t][:, m:m + 1])
                ep1 = spool.tile([128, 4, 128], F32, name=f"ep1_{j}_{t}", tag="ep1")
                nc.vector.tensor_scalar(ep1[:, :, :], eneg[:, :, :], 1e9, 1.0,
                                        op0=AOP.min, op1=AOP.add)
                rza = spool.tile([128, 4, 128], BF16, name=f"rza_{j}_{t}", tag="rza")
                with nc.allow_low_precision(reason="sigmoid gates in (0,1), bf16 ok"):
                    nc.vector.reciprocal(rza[:, :, :], ep1[:, :, :])
                tmp = scrpool.tile([128, 2, 128], F32, name=f"tmp_{j}_{t}", tag="tmp")
                for m in range(2):
                    nc.vector.tensor_scalar(tmp[:, m, :], hnps[:, m, :],
                                            bnhc_sb[t][:, m:m + 1], None,
                                            op0=AOP.add)
                tmp2 = scrpool.tile([128, 2, 128], F32, name=f"tmp2_{j}_{t}", tag="tmp2")
                nc.vector.tensor_tensor(tmp2[:, :, :], rza[:, 0:2, :],
                                        tmp[:, :, :], op=AOP.mult)
                t2 = scrpool.tile([128, 2, 128], F32, name=f"t2_{j}_{t}", tag="t2")
                nc.vector.tensor_tensor(t2[:, :, :], tmp2[:, :, :],
                                        inps[:, :, :], op=AOP.add)
                nn = spool.tile([128, 2, 128], BF16, name=f"nn_{j}_{t}", tag="nn")
                for m in range(2):
                    nc.scalar.activation(nn[:, m, :], t2[:, m, :], ACT.Tanh,
                                         bias=bnxc_sb[t][:, m:m + 1])
                yield
                hm = scrpool.tile([128, 2, 128], BF16, name=f"hm_{j}_{t}", tag="hm")
                nc.vector.tensor_tensor(hm[:, :, :], hTb[:, :, :], nn[:, :, :],
                                        op=AOP.subtract)
                hz = scrpool.tile([128, 2, 128], BF16, name=f"hz_{j}_{t}", tag="hz")
                nc.vector.tensor_tensor(hz[:, :, :], hm[:, :, :],
                                        rza[:, 2:4, :], op=AOP.mult)
                gf_new = spool.tile([128, 2, 128], F32, name=f"gfn_{j}_{t}", tag="gf", bufs=4)
                nc.vector.tensor_tensor(gf_new[:, :, :], hz[:, :, :],
                                        nn[:, :, :], op=AOP.add)
                gf = gf_new
                if t == 0:
                    hTb_new = spool.tile([128, 2, 128], BF16, name=f"hTbn_{j}_{t}", tag="hTb", bufs=4)
                    nc.vector.tensor_copy(hTb_new[:, :, :], gf[:, :, :])
                    hTb = hTb_new
            yield
            for k in range(2):
                nc.sync.dma_start(out_d[j * 256 + k * 128: j * 256 + (k + 1) * 128, :],
                                  gf[:, k, :])

        INTERLEAVE = 3
        active = []
        nxt = 0
        while active or nxt < NT_G:
            while len(active) < INTERLEAVE and nxt < NT_G:
                active.append(tile_work(nxt))
                nxt += 1
            for g in list(active):
                try:
                    next(g)
                except StopIteration:
                    active.remove(g)
    nc.finalize()
    return nc, ctx


def _window_offsets(seg, bounds_g):
    """Global per-subtile window offsets; assert W covers all spans."""
    lo = {}
    hi = {}
    for c in range(NCORES):
        for gt in range(bounds_g[c], bounds_g[c + 1], 128):
            ge = min(gt + 128, bounds_g[c + 1])
            a = int(np.searchsorted(seg, gt, 'left'))
            b = int(np.searchsorted(seg, ge, 'left'))
            for s in range((b - a + 127) // 128):
                s0, s1 = a + s * 128, min(a + (s + 1) * 128, b)
                rlo, rhi = int(seg[s0] - gt), int(seg[s1 - 1] - gt)
                lo[s] = min(lo.get(s, 1 << 30), rlo)
                hi[s] = max(hi.get(s, -1), rhi)
    nsub = max(lo) + 1
    offs = []
    for s in range(nsub):
        o = min(max(0, lo[s]), 128 - W)
        assert hi[s] < o + W, f"window W={W} too small at s={s}: [{lo[s]},{hi[s]}]"
        offs.append(o)
    return offs


def _prep_core(node_feats_bf, node_feats_f32, seg, g_lo, g_hi, NT_G, NSUB, offs):
    nf = np.zeros((NT_G * 128, NSUB * F), NP_BF16)
    nt8 = np.zeros((NT_G * 128, 2 * NSUB * 128), NP_FP8)
    segw = np.full((NT_G * 128, NSUB), -1.0, np.float32)
    for j in range(NT_G):
        gt = g_lo + j * 128
        if gt >= g_hi:
            continue
        ge = min(gt + 128, g_hi)
        a = int(np.searchsorted(seg, gt, 'left'))
        b = int(np.searchsorted(seg, ge, 'left'))
        for s in range((b - a + 127) // 128):
            s0 = a + s * 128
            s1 = min(s0 + 128, b)
            n = s1 - s0
            blk = node_feats_bf[s0:s1]  # [n, 256] bf16
            nf[j * 128: j * 128 + n, s * F:(s + 1) * F] = blk
            # transposed fp8: rows = f (0..127), col k*NSUB*128 + s*128 + p
            blk8 = node_feats_f32[s0:s1].astype(NP_FP8)  # [n, 256]
            for k in range(2):
                nt8[j * 128:(j + 1) * 128,
                    k * NSUB * 128 + s * 128: k * NSUB * 128 + s * 128 + n] = \
                    blk8[:, k * 128:(k + 1) * 128].T
            segw[j * 128: j * 128 + n, s] = (seg[s0:s1] - gt - offs[s]).astype(np.float32)
    return nf, nt8.reshape(NT_G * 128, -1), segw.astype(NP_BF16)


def kernel(node_feats, seg_ids, Wl, bl, Wp, bp, Wih, Whh, bih, bhh):
    node_feats = np.asarray(node_feats, np.float32)
    seg = np.asarray(seg_ids).astype(np.int64)
    Wl = np.asarray(Wl, np.float32)
    bl = np.asarray(bl, np.float32)
    Wp = np.asarray(Wp, np.float32)
    bp = np.asarray(bp, np.float32)
    Wih = np.asarray(Wih, np.float32)
    Whh = np.asarray(Whh, np.float32)
    bih = np.asarray(bih, np.float32)
    bhh = np.asarray(bhh, np.float32)
    V = node_feats.shape[0]
    G = 25000

    bounds_g = [0]
    for c in range(1, NCORES):
        bounds_g.append(int(seg[c * V // NCORES]))
    bounds_g.append(G)
    NT_G = max((bounds_g[c + 1] - bounds_g[c] + 127) // 128 for c in range(NCORES))

    offs = _window_offsets(seg, bounds_g)
    maxnodes = 0
    for c in range(NCORES):
        for gt in range(bounds_g[c], bounds_g[c + 1], 128):
            ge = min(gt + 128, bounds_g[c + 1])
            a = np.searchsorted(seg, gt, 'left')
            b = np.searchsorted(seg, ge, 'left')
            maxnodes = max(maxnodes, int(b - a))
    NSUB = (maxnodes + 127) // 128
    assert len(offs) <= NSUB
    offs = offs + [0] * (NSUB - len(offs))

    nc, ctx = _build_program(NT_G, NSUB, offs)

    NPAD = 1 if (NSUB + 1) * W % 16 == 0 else 2
    NIDX = (NSUB + NPAD) * W
    # ap_gather wrapped indices: flat position i -> offs[i // W] + i % W
    unw = np.zeros(NIDX, np.int16)
    for i in range(NSUB * W):
        unw[i] = offs[i // W] + (i % W)
    gidx = np.zeros((128, NIDX // 16), np.int16)
    for grp in range(8):
        for p in range(16):
            for col in range(NIDX // 16):
                gidx[grp * 16 + p, col] = unw[col * 16 + p]

    bih_eff = bih - Wih.sum(axis=2)  # fold ELU's -1 shift (x-side)
    shared = {
        "gidx": gidx,
        "iotaw": np.broadcast_to(np.arange(W, dtype=np.float32), (128, W)).astype(NP_BF16),
        "onesr": np.ones((1, 128), NP_BF16),
        "onescol": np.ones((128, 1), NP_BF16),
        "zrow": np.zeros((1, 128), NP_BF16),
        "zcol128": np.zeros((1, 128), NP_BF16),
        "wl8": np.stack([Wl[:, 0, F + k * 128:F + (k + 1) * 128] for k in range(2)],
                        axis=0).transpose(2, 0, 1).reshape(128, 4).astype(NP_FP8),
        "wlg": np.stack([Wl[:, 0, k * 128:(k + 1) * 128] for k in range(2)],
                        axis=0).transpose(2, 0, 1).reshape(128, 4).astype(NP_BF16),
        "blrow": np.repeat(bl[:, 0], 128)[None, :].astype(NP_BF16),
    }
    for t in range(T):
        wpt = np.zeros((128, 4 * 128), np.float32)
        WpT = Wp[t].T  # [f_in, f_out]
        for k in range(2):
            for m in range(2):
                wpt[:, (k * 2 + m) * 128:(k * 2 + m + 1) * 128] = \
                    WpT[k * 128:(k + 1) * 128, m * 128:(m + 1) * 128]
        shared[f"wpt{t}"] = wpt.astype(NP_BF16)
        shared[f"bpc{t}"] = bp[t].reshape(2, 128).T.copy().astype(np.float32)
        for nm, Wm in (("wih", Wih[t]), ("whh", Whh[t])):
            wt = np.zeros((128, 12 * 128), np.float32)
            WT = Wm.T.copy()  # [f(256), c(768)]
            WT[:, :512] = -WT[:, :512]  # r/z rows negated: sigmoid via exp
            for k in range(2):
                for m in range(6):
                    wt[:, (k * 6 + m) * 128:(k * 6 + m + 1) * 128] = \
                        WT[k * 128:(k + 1) * 128, m * 128:(m + 1) * 128]
            shared[f"{nm}{t}"] = wt.astype(NP_BF16)
        shared[f"brzc{t}"] = (-(bih_eff[t][:512] + bhh[t][:512])).reshape(4, 128).T.copy().astype(np.float32)
        shared[f"bnxc{t}"] = bih_eff[t][512:].reshape(2, 128).T.copy().astype(np.float32)
        shared[f"bnhc{t}"] = bhh[t][512:].reshape(2, 128).T.copy().astype(np.float32)

    nf_bf = node_feats.astype(NP_BF16)
    in_maps = []
    for c in range(NCORES):
        nf, nt8, segw = _prep_core(nf_bf, node_feats, seg,
                                   bounds_g[c], bounds_g[c + 1], NT_G, NSUB, offs)
        m = dict(shared)
        m["nf"] = nf
        m["nt8"] = nt8
        m["segw"] = segw
        in_maps.append(m)

    res = run_bass_kernel_spmd(nc, in_maps, core_ids=list(range(NCORES)))
    ctx.close()
    global LAST_RESULT
    LAST_RESULT = res

    out = np.zeros((G, F), np.float32)
    for c in range(NCORES):
        arr = res.results[c]["out"].reshape(NT_G, 2, 128, 128)
        gc = bounds_g[c + 1] - bounds_g[c]
        full = arr.transpose(0, 3, 1, 2).reshape(NT_G * 128, 256)
        out[bounds_g[c]:bounds_g[c + 1]] = full[:gc]
    return out
